# revision 1
# baseline (speedup 1.0000x reference)
"""DBPNet Trainium2 kernel: 8-core data-parallel Bass/Tile implementation.

Scheme (validated by layout_sim.py):
  - batch-major state [32, N]: row = chan*16 + s (16 samples/core)
  - complex matmuls are "state-stationary": lhsT = combo(state) [2K-chunked, 32],
    rhs = replicated matrix stacks streamed as the moving operand (fp32r)
  - comboH (A^H-type):  top [s_r|s_i], bottom [s_i|-s_r]
  - comboN (C*s-type):  top [s_r|s_i], bottom [-s_i|s_r]
  - AAH = A A^H precomputed on host =>  Atx = arc + rho*c1*AAH*zmu,
    Ax = Atx - AAH*tmv  (A-matmuls only once per iteration)
  - CNN in (co*4+q, (b', l)) layout with block-diagonal weights
  - BatchNorm batch stats made exact across cores via AllReduce
"""
import numpy as np

B, Nv, Nt, F = 128, 512, 2048, 32
NCORE, BS = 8, 16
ITERS, ADMM = 5, 3
BN_EPS = 1e-5


# ---------------------------------------------------------------- host prep
def _host_prep(inputs):
    A = np.ascontiguousarray(np.asarray(inputs['A'], np.float32))
    Ar, Ai = A[0], A[1]
    Ac = Ar.astype(np.float64) + 1j * Ai.astype(np.float64)
    AAH = Ac @ Ac.conj().T
    AAHr = AAH.real.astype(np.float32)
    AAHi = AAH.imag.astype(np.float32)

    rhos = np.exp(np.asarray(inputs['log_rho'], np.float32)).astype(np.float32)
    epss = np.exp(np.asarray(inputs['log_eps'], np.float32)).astype(np.float32)

    minv_stacks, rho_to_idx, iter_minv_idx = [], {}, []
    for r in rhos:
        key = float(r)
        if key not in rho_to_idx:
            M = np.linalg.inv(AAH + key * np.eye(Nv))
            Mr = M.real.astype(np.float32)
            Mi = M.imag.astype(np.float32)
            minv_stacks.append(
                np.concatenate([Mr.T, Mi.T], 0).reshape(8, 128, 512)
                .transpose(1, 0, 2).copy())            # [128, 8, 512]
            rho_to_idx[key] = len(minv_stacks) - 1
        iter_minv_idx.append(rho_to_idx[float(r)])

    A1 = np.concatenate([Ar, Ai], 0)                    # [1024, 2048]
    AB = A1.reshape(8, 128, 2048).transpose(1, 0, 2).copy()   # [128, 8, 2048]
    AT1 = np.concatenate([Ar.T, Ai.T], 0)               # [4096, 512]
    ATD = AT1.reshape(32, 128, 512).copy()              # [32][128, 512]
    AAH1 = np.concatenate([AAHr.T, AAHi.T], 0)          # [1024, 512]
    AAHD = AAH1.reshape(8, 128, 512).transpose(1, 0, 2).copy()  # [128, 8, 512]

    w1 = np.asarray(inputs['conv1_w'], np.float32)
    w2 = np.asarray(inputs['conv2_w'], np.float32)
    wf = np.asarray(inputs['convf_w'], np.float32)
    W1 = np.zeros((128, 128), np.float32)
    for dl in range(3):
        for ci in range(2):
            for q in range(4):
                W1[dl * 8 + ci * 4 + q, np.arange(F) * 4 + q] = w1[:, ci, dl]
    W2 = np.zeros((3, 128, 128), np.float32)
    WFm = np.zeros((3, 128, 8), np.float32)
    for dl in range(3):
        for ci in range(F):
            for q in range(4):
                W2[dl, ci * 4 + q, np.arange(F) * 4 + q] = w2[:, ci, dl]
                WFm[dl, ci * 4 + q, np.arange(2) * 4 + q] = wf[:, ci, dl]

    onesel = np.zeros((128, 32), np.float32)
    selback = np.zeros((128, 128), np.float32)   # rows 0-31 used
    for co in range(32):
        for q in range(4):
            onesel[co * 4 + q, co] = 1.0
            selback[co, co * 4 + q] = 1.0
    ident32 = np.zeros((128, 32), np.float32)
    ident32[:32, :32] = np.eye(32, dtype=np.float32)
    ones11 = np.zeros((128, 1), np.float32)
    ones11[0, 0] = 1.0

    # WTS pack [128, 128 + 384 + 24 + 32 + 128 + 32 + 1]
    WTS = np.concatenate(
        [W1] + [W2[d] for d in range(3)] + [WFm[d] for d in range(3)]
        + [onesel, selback, ident32, ones11], axis=1)

    g1 = np.asarray(inputs['bn1_g'], np.float32)
    b1 = np.asarray(inputs['bn1_b'], np.float32)
    g2 = np.asarray(inputs['bn2_g'], np.float32)
    b2 = np.asarray(inputs['bn2_b'], np.float32)
    fb = np.asarray(inputs['convf_b'], np.float32)
    CF = np.zeros((128, 8), np.float32)
    CF[:32, 0] = g1
    CF[:32, 1] = b1
    CF[:32, 2] = g2
    CF[:32, 3] = b2
    CF[:16, 4] = fb[0]
    CF[16:32, 4] = fb[1]
    CF[:, 6] = BN_EPS   # col 5 stays zero (zero-bias AP)

    y = np.asarray(inputs['y'], np.float32)
    ybm_cores, ylhsT_cores = [], []
    for c in range(NCORE):
        ys = y[c * BS:(c + 1) * BS]
        ybm = np.concatenate([ys[:, 0], ys[:, 1]], 0)    # [32, Nv]
        ybm_cores.append(np.ascontiguousarray(ybm))
        sT = ybm.T                                       # [Nv, 32]
        bot = np.concatenate([sT[:, 16:], -sT[:, :16]], 1)
        comboH = np.concatenate([sT, bot], 0)            # [2Nv, 32]
        ylhsT_cores.append(
            comboH.reshape(8, 128, 32).transpose(1, 0, 2).copy())  # [128, 8, 32]

    return dict(AB=AB, ATD=ATD, AAHD=AAHD, minv_stacks=minv_stacks,
                iter_minv_idx=iter_minv_idx, rhos=rhos, epss=epss,
                WTS=WTS, CF=CF, ybm_cores=ybm_cores, ylhsT_cores=ylhsT_cores)


# WTS column offsets
W1_C = 0
W2_C = 128
WF_C = 128 + 384
OSEL_C = WF_C + 24
SELB_C = OSEL_C + 32
ID32_C = SELB_C + 128
ONE1_C = ID32_C + 32
WTS_W = ONE1_C + 1


# ---------------------------------------------------------------- program
def _build_program(prep):
    import concourse.bacc as bacc
    import concourse.tile as tile
    import concourse.mybir as mybir

    dt = mybir.dt
    f32, f32r = dt.float32, dt.float32r
    AX = mybir.AxisListType
    OP = mybir.AluOpType
    AF = mybir.ActivationFunctionType

    nu = len(prep['minv_stacks'])
    rhos, epss = prep['rhos'], prep['epss']
    cnt = float(B * Nt)

    nc = bacc.Bacc("TRN2", target_bir_lowering=False, debug=False,
                   num_devices=NCORE)

    AB_d = nc.dram_tensor("AB", [128, 8, 2048], f32r, kind="ExternalInput")
    AT_d = nc.dram_tensor("ATD", [32, 128, 512], f32r, kind="ExternalInput")
    AAH_d = nc.dram_tensor("AAHD", [128, 8, 512], f32r, kind="ExternalInput")
    MINV_d = nc.dram_tensor("MINVS", [nu, 128, 8, 512], f32r, kind="ExternalInput")
    WTS_d = nc.dram_tensor("WTS", [128, WTS_W], f32r, kind="ExternalInput")
    CF_d = nc.dram_tensor("CF", [128, 8], f32, kind="ExternalInput")
    Y_d = nc.dram_tensor("YBM", [32, 512], f32r, kind="ExternalInput")
    YL_d = nc.dram_tensor("YL", [128, 8, 32], f32r, kind="ExternalInput")
    XO_d = nc.dram_tensor("XOUT", [32, 2048], f32r, kind="ExternalOutput")
    DBG_d = nc.dram_tensor("DBG", [32, 2048], f32r, kind="ExternalOutput")
    DBG2_d = nc.dram_tensor("DBG2", [128, 4096], f32r, kind="ExternalOutput")
    DBG4_d = nc.dram_tensor("DBG4", [32, 3, 2048], f32r, kind="ExternalOutput")
    DBG3_d = nc.dram_tensor("DBG3", [32, 4096], f32r, kind="ExternalOutput")

    with tile.TileContext(nc) as tc:
        with (
            tc.tile_pool(name="cst", bufs=1) as cst,
            tc.tile_pool(name="atp", bufs=4) as atp,
            tc.tile_pool(name="st", bufs=1) as stp,
            tc.tile_pool(name="cmb", bufs=1) as cmb,
            tc.tile_pool(name="act", bufs=5) as actp,
            tc.tile_pool(name="xin", bufs=1) as xinp,
            tc.tile_pool(name="sc", bufs=1) as scp,
            tc.tile_pool(name="psA", bufs=3, space="PSUM") as psA,
            tc.tile_pool(name="psB", bufs=2, space="PSUM") as psB,
            tc.tile_pool(name="psC", bufs=1, space="PSUM") as psC,
            tc.tile_pool(name="psD", bufs=2, space="PSUM") as psD,
            tc.tile_pool(name="ddr", bufs=2, space="DRAM") as ddr,
        ):
            # ---- constants into SBUF ----
            ab = cst.tile([128, 8, 2048], f32r, tag="ab")
            aah = cst.tile([128, 8, 512], f32r, tag="aah")
            minv = cst.tile([128, 8, 512], f32r, tag="minv")
            wts = cst.tile([128, WTS_W], f32r, tag="wts")
            cf = cst.tile([128, 8], f32, tag="cf")
            yl = cst.tile([128, 8, 32], f32r, tag="yl")
            nc.sync.dma_start(ab[:], AB_d[:])
            nc.sync.dma_start(aah[:], AAH_d[:])
            nc.sync.dma_start(wts[:], WTS_d[:])
            nc.sync.dma_start(cf[:], CF_d[:])
            nc.sync.dma_start(yl[:], YL_d[:])
            if nu == 1:
                nc.sync.dma_start(minv[:], MINV_d[0])

            W1 = wts[:, W1_C:W1_C + 128]
            W2 = [wts[:, W2_C + 128 * d: W2_C + 128 * (d + 1)] for d in range(3)]
            WF = [wts[:, WF_C + 8 * d: WF_C + 8 * (d + 1)] for d in range(3)]
            OSEL = wts[:, OSEL_C:OSEL_C + 32]
            SELB = wts[0:32, SELB_C:SELB_C + 128]
            ID32 = wts[0:32, ID32_C:ID32_C + 32]
            ONE1 = wts[0:1, ONE1_C:ONE1_C + 1]
            g32 = [cf[0:32, 0:1], cf[0:32, 2:3]]
            b32 = [cf[0:32, 1:2], cf[0:32, 3:4]]
            fb32 = cf[0:32, 4:5]
            zb128 = cf[:, 5:6]
            zb32 = cf[0:32, 5:6]
            zb1 = cf[0:1, 5:6]
            epsb = cf[0:32, 6:7]

            # ---- state ----
            X2 = stp.tile([32, 2, 2048], f32r, tag="X2")     # 0: x, 1: rc1
            x_t = X2[:, 0, :]
            rc1_t = X2[:, 1, :]
            S = stp.tile([32, 8, 512], f32r, tag="S")
            z_t, u_t, y_t = S[:, 0, :], S[:, 1, :], S[:, 2, :]
            atx_t, tmv_t = S[:, 3, :], S[:, 4, :]
            v_t, dif_t, arc_t = S[:, 5, :], S[:, 6, :], S[:, 7, :]
            sq_t = stp.tile([32, 512], f32r, tag="sq")
            s32f = stp.tile([32, 1], f32, tag="s32f")
            nc.sync.dma_start(y_t[:], Y_d[:])
            nc.vector.memset(u_t[:].bitcast(f32), 0.0)

            zmuT = cmb.tile([128, 12, 32], f32r, tag="zmuT")
            atxT = cmb.tile([128, 12, 32], f32r, tag="atxT")
            tmvT = cmb.tile([128, 12, 32], f32r, tag="tmvT")
            rcT = cmb.tile([128, 32, 32], f32r, tag="rcT")
            smal = cmb.tile([32, 8], f32r, tag="smal")
            gb_t = smal[:, 1:3]
            mean_t, var_t = smal[:, 3:4], smal[:, 4:5]
            ssn_t, m2_t = smal[:, 5:6], smal[:, 6:7]
            row1 = cmb.tile([1, 96], f32, tag="row1")       # factor chain
            n2_t = row1[:, 0:16]
            nrm_t = row1[:, 16:32]
            fac_t = row1[:, 32:48]
            facd_t = row1[:, 48:64]  # unused half; facd uses 32 cols below
            gbb = cmb.tile([128, 2], f32, tag="gbb")
            stat = cmb.tile([128, 4, 16], f32, tag="stat")  # sums/sumsq per tile
            stat2 = cmb.tile([128, 2], f32, tag="stat2")
            stat2r = cmb.tile([128, 2], f32r, tag="stat2r")

            def combo_build(dst, src_bm, nchunk, kinds):
                """dst [128, 12|32, 32]; src_bm [32, nchunk*128]; kinds subset
                of {'H','N'}. chunks: 0..nc-1 top; nc..2nc-1 H-bot or N-bot at
                fixed offsets: H at nchunk, N at 2*nchunk (for 12-layout: 4/8)."""
                for c in range(nchunk):
                    pT = psD.tile([128, 32], f32r, tag="tp")
                    nc.tensor.transpose(pT[:], src_bm[:, 128 * c:128 * (c + 1)],
                                        ID32)
                    nc.vector.tensor_copy(dst[:, c, :], pT[:])
                    if 'H' in kinds:
                        o = nchunk
                        nc.vector.tensor_copy(dst[:, o + c, 0:16],
                                              pT[:, 16:32])
                        nc.vector.tensor_scalar_mul(dst[:, o + c, 16:32],
                                                    pT[:, 0:16], -1.0)
                    if 'N' in kinds:
                        o = nchunk if nchunk == 16 else 2 * nchunk
                        nc.vector.tensor_scalar_mul(dst[:, o + c, 0:16],
                                                    pT[:, 16:32], -1.0)
                        nc.vector.tensor_copy(dst[:, o + c, 16:32],
                                              pT[:, 0:16])

            def mm_chunks(psum, lhsT_tile, rhs, kmap, n0, nn):
                """psum [32, nn] += sum_k lhsT[:,kmap[k],:].T @ rhs[:,k,n0:n0+nn]"""
                nk = len(kmap)
                for ki, kc in enumerate(kmap):
                    nc.tensor.matmul(psum[:], lhsT_tile[:, kc, :],
                                     rhs[:, ki, n0:n0 + nn],
                                     start=(ki == 0), stop=(ki == nk - 1))

            KH = [0, 1, 2, 3, 4, 5, 6, 7]          # comboH chunks in 12-layout
            KN = [0, 1, 2, 3, 8, 9, 10, 11]        # comboN chunks in 12-layout

            # ---- x0 = A^H y ----
            for nt in range(4):
                p = psA.tile([32, 512], f32, tag="mm")
                for k in range(8):
                    nc.tensor.matmul(p[:], yl[:, k, :], ab[:, k, 512 * nt:512 * (nt + 1)],
                                     start=(k == 0), stop=(k == 7))
                nc.vector.tensor_copy(x_t[:, 512 * nt:512 * (nt + 1)], p[:])

            # ================= iterations =================
            for it in range(ITERS):
                rho = float(rhos[it])
                eps = float(epss[it])
                c1 = 1.0 / (rho + 1e-8)
                if nu > 1:
                    nc.sync.dma_start(minv[:], MINV_d[prep['iter_minv_idx'][it]])

                # ---------- CNN ----------
                act1 = []
                for bp in range(4):
                    xin = xinp.tile([32, 2048], f32r, tag="xin")
                    nc.vector.memset(xin[:, 0:1].bitcast(f32), 0.0)
                    nc.vector.memset(xin[:, 2047:2048].bitcast(f32), 0.0)
                    for dl in range(3):
                        lo, hi = max(0, 1 - dl), min(2048, 2048 + 1 - dl)
                        for ci in range(2):
                            src = x_t[ci * 16 + bp * 4: ci * 16 + bp * 4 + 4,
                                      lo + dl - 1: hi + dl - 1]
                            nc.sync.dma_start(
                                xin[dl * 8 + ci * 4: dl * 8 + ci * 4 + 4, lo:hi], src)
                    a1 = actp.tile([128, 2050], f32r, tag="act")
                    nc.vector.memset(a1[:, 0:1].bitcast(f32), 0.0)
                    nc.vector.memset(a1[:, 2049:2050].bitcast(f32), 0.0)
                    for lt in range(4):
                        p = psB.tile([128, 512], f32, tag="big")
                        nc.tensor.matmul(p[:], W1[0:24, :],
                                         xin[0:24, 512 * lt:512 * (lt + 1)],
                                         start=True, stop=True)
                        idx = bp * 4 + lt
                        nc.vector.tensor_copy(
                            a1[:, 1 + 512 * lt:1 + 512 * (lt + 1)], p[:])
                        sj = scp.tile([128, 512], f32, tag="sqj")
                        nc.scalar.activation(
                            sj[:], p[:], AF.Square, bias=zb128[:],
                            accum_out=stat[:, idx // 4, 4 + idx % 4:5 + idx % 4])
                    nc.vector.tensor_reduce(stat[:, bp, 8:9],
                                            a1[:, 1:2049], AX.X, OP.add)
                    act1.append(a1)

                def bn_apply(layer, acts):
                    # stat[:, t, 0:4] sums, [:, t, 4:8] sumsq for 4 l-tiles
                    with nc.allow_low_precision(reason="f32r rounding of fp32 sums"):
                        nc.vector.tensor_reduce(stat2[:, 0:1], stat[:, :, 8:9],
                                                AX.XY, OP.add)
                        nc.vector.tensor_reduce(stat2[:, 1:2], stat[:, :, 4:8],
                                                AX.XY, OP.add)
                    ci_ = ddr.tile([128, 2], f32, tag="cc")
                    co_ = ddr.tile([128, 2], f32, tag="cc")
                    nc.sync.dma_start(ci_[:], stat2[:])
                    nc.gpsimd.collective_compute(
                        "AllReduce", OP.add, replica_groups=[list(range(NCORE))],
                        ins=[ci_.opt()], outs=[co_.opt()])
                    nc.sync.dma_start(stat2[:], co_[:])
                    nc.vector.tensor_copy(stat2r[:], stat2[:])
                    p = psC.tile([32, 2], f32, tag="sm")
                    nc.tensor.matmul(p[:], OSEL, stat2r[:],
                                     start=True, stop=True)
                    with nc.allow_low_precision(reason="bn scalar math in f32r"):
                        nc.vector.tensor_scalar_mul(mean_t[:], p[:, 0:1], 1.0 / cnt)
                        nc.vector.tensor_scalar_mul(ssn_t[:], p[:, 1:2], 1.0 / cnt)
                        nc.vector.tensor_mul(m2_t[:], mean_t[:], mean_t[:])
                        nc.vector.tensor_sub(var_t[:], ssn_t[:], m2_t[:])
                        nc.scalar.activation(var_t[:], var_t[:], AF.Sqrt, bias=epsb[:])
                        nc.vector.reciprocal(var_t[:], var_t[:])
                        nc.vector.tensor_mul(gb_t[:, 0:1], g32[layer][:], var_t[:])
                        nc.vector.tensor_mul(m2_t[:], mean_t[:], gb_t[:, 0:1])
                        nc.vector.tensor_sub(gb_t[:, 1:2], b32[layer][:], m2_t[:])
                    p2 = psC.tile([128, 2], f32, tag="sm")
                    nc.tensor.matmul(p2[:], SELB, gb_t[:],
                                     start=True, stop=True)
                    nc.vector.tensor_copy(gbb[:], p2[:])
                    for a in acts:
                        nc.scalar.activation(a[:, 1:2049], a[:, 1:2049], AF.Relu,
                                             bias=gbb[:, 1:2], scale=gbb[:, 0:1])

                bn_apply(0, act1)
                if it == 0:
                    nc.sync.dma_start(DBG2_d[:, 0:2048], act1[0][:, 1:2049])

                # conv2
                act2 = []
                for bp in range(4):
                    a2 = actp.tile([128, 2050], f32r, tag="act")
                    nc.vector.memset(a2[:, 0:1].bitcast(f32), 0.0)
                    nc.vector.memset(a2[:, 2049:2050].bitcast(f32), 0.0)
                    for lt in range(4):
                        p = psB.tile([128, 512], f32, tag="big")
                        for dl in range(3):
                            nc.tensor.matmul(
                                p[:], W2[dl],
                                act1[bp][:, dl + 512 * lt: dl + 512 * (lt + 1)],
                                start=(dl == 0), stop=(dl == 2))
                        idx = bp * 4 + lt
                        nc.vector.tensor_copy(
                            a2[:, 1 + 512 * lt:1 + 512 * (lt + 1)], p[:])
                        sj = scp.tile([128, 512], f32, tag="sqj")
                        nc.scalar.activation(
                            sj[:], p[:], AF.Square, bias=zb128[:],
                            accum_out=stat[:, idx // 4, 4 + idx % 4:5 + idx % 4])
                    nc.vector.tensor_reduce(stat[:, bp, 8:9],
                                            a2[:, 1:2049], AX.X, OP.add)
                    act2.append(a2)

                bn_apply(1, act2)
                if it == 0:
                    nc.sync.dma_start(DBG2_d[:, 2048:4096], act2[0][:, 1:2049])

                # convf + residual: rc1 = (x + convf + fb) * c1
                for bp in range(4):
                    for lt in range(4):
                        p = psC.tile([8, 512], f32, tag="sm")
                        for dl in range(3):
                            nc.tensor.matmul(
                                p[:], WF[dl],
                                act2[bp][:, dl + 512 * lt: dl + 512 * (lt + 1)],
                                start=(dl == 0), stop=(dl == 2))
                        s8 = scp.tile([8, 512], f32r, tag="s8")
                        nc.vector.tensor_copy(s8[:], p[:])
                        for cfi in range(2):
                            nc.sync.dma_start(
                                rc1_t[cfi * 16 + bp * 4: cfi * 16 + bp * 4 + 4,
                                      512 * lt:512 * (lt + 1)],
                                s8[cfi * 4:cfi * 4 + 4, :])
                nc.vector.tensor_add(rc1_t[:], rc1_t[:], x_t[:])
                nc.scalar.activation(rc1_t[:], rc1_t[:], AF.Identity,
                                     bias=fb32[:], scale=c1)
                if it == 1:
                    nc.sync.dma_start(DBG_d[:], rc1_t[:])


                # arc = A * rc1 (comboN(rc1) against AT chunks, streamed)
                combo_build(rcT, rc1_t, 16, {'N'})
                parc = psA.tile([32, 512], f32, tag="mm")
                for k in range(32):
                    atc = atp.tile([128, 512], f32r, tag="atc")
                    nc.sync.dma_start(atc[:], AT_d[k])
                    kc = k if k < 16 else k  # rcT: top 0-15, N-bot 16-31
                    nc.tensor.matmul(parc[:], rcT[:, kc, :],
                                     atc[:], start=(k == 0), stop=(k == 31))
                nc.vector.tensor_copy(arc_t[:], parc[:])

                # ---------- ADMM ----------
                for s in range(ADMM):
                    final = (s == ADMM - 1)
                    # zmu
                    zsrc = y_t if s == 0 else z_t
                    nc.vector.tensor_sub(sq_t[:], zsrc[:], u_t[:])  # sq_t = zmu
                    combo_build(zmuT, sq_t, 4, {'N', 'H'} if final else {'N'})
                    # aahz -> Atx = arc + rho*c1*aahz
                    p = psA.tile([32, 512], f32, tag="mm")
                    mm_chunks(p, zmuT, aah, KN, 0, 512)
                    nc.vector.scalar_tensor_tensor(atx_t[:], p[:], rho * c1,
                                                   arc_t[:], OP.mult, OP.add)
                    if it == 0:
                        nc.sync.dma_start(DBG3_d[:, 1024 * s:1024 * s + 512], atx_t[:])
                    combo_build(atxT, atx_t, 4, {'N'})
                    # tmv = Minv * Atx
                    p = psA.tile([32, 512], f32, tag="mm")
                    mm_chunks(p, atxT, minv, KN, 0, 512)
                    nc.vector.tensor_copy(tmv_t[:], p[:])
                    combo_build(tmvT, tmv_t, 4, {'N', 'H'} if final else {'N'})
                    # Ax = Atx - AAH*tmv
                    p = psA.tile([32, 512], f32, tag="mm")
                    mm_chunks(p, tmvT, aah, KN, 0, 512)
                    nc.vector.tensor_sub(v_t[:], atx_t[:], p[:])   # v_t = Ax
                    if final:
                        for nt in range(4):
                            p = psA.tile([32, 512], f32, tag="mm")
                            mm_chunks(p, zmuT, ab, KH, 512 * nt, 512)
                            nc.vector.scalar_tensor_tensor(
                                x_t[:, 512 * nt:512 * (nt + 1)], p[:], rho * c1,
                                rc1_t[:, 512 * nt:512 * (nt + 1)], OP.mult, OP.add)
                        for nt in range(4):
                            p = psA.tile([32, 512], f32, tag="mm")
                            mm_chunks(p, tmvT, ab, KH, 512 * nt, 512)
                            nc.vector.tensor_sub(x_t[:, 512 * nt:512 * (nt + 1)],
                                                 x_t[:, 512 * nt:512 * (nt + 1)], p[:])
                    # projection
                    nc.vector.tensor_add(v_t[:], v_t[:], u_t[:])   # v = Ax + u
                    nc.vector.tensor_sub(dif_t[:], v_t[:], y_t[:])
                    nc.vector.scalar_tensor_tensor(sq_t[:], dif_t[:], 1.0,
                                                   dif_t[:], OP.mult, OP.mult,
                                                   accum_out=s32f[:])
                    pt = psC.tile([1, 32], f32, tag="sm")
                    nc.tensor.matmul(pt[:], s32f[:], ID32.bitcast(f32),
                                     is_transpose=True)
                    nc.vector.tensor_copy(row1[:, 64:96], pt[:])
                    nc.vector.tensor_add(n2_t[:], row1[:, 64:80], row1[:, 80:96])
                    nc.scalar.activation(nrm_t[:], n2_t[:], AF.Sqrt, bias=zb1[:])
                    nc.vector.tensor_scalar_add(nrm_t[:], nrm_t[:], 1e-12)
                    nc.vector.reciprocal(nrm_t[:], nrm_t[:])
                    nc.vector.tensor_scalar_mul(fac_t[:], nrm_t[:], eps)
                    nc.vector.tensor_scalar_min(fac_t[:], fac_t[:], 1.0)
                    fr = psC.tile([32, 1], f32, tag="sm")
                    nc.vector.tensor_copy(row1[:, 48:64], fac_t[:])
                    nc.tensor.matmul(fr[:], row1[:, 32:64], ONE1.bitcast(f32),
                                     is_transpose=True)
                    nc.vector.scalar_tensor_tensor(z_t[:], dif_t[:], fr[:],
                                                   y_t[:], OP.mult, OP.add)
                    nc.vector.tensor_sub(u_t[:], v_t[:], z_t[:])
                    if it == 0:
                        nc.sync.dma_start(DBG3_d[:, 1024 * s + 512:1024 * (s + 1)], z_t[:])

                if 1 <= it <= 3:
                    nc.sync.dma_start(DBG4_d[:, it - 1, :], x_t[:])

            nc.sync.dma_start(XO_d[:], x_t[:])

    nc.compile()
    return nc


_CACHE = {}


def _enable_trace_shim():
    import sys, types
    try:
        import trn_agent_boot.trn_boot as _tb
        import concourse.bass_utils as _bu
        _bu.upload_artifacts = lambda tmpdir: "local://" + str(tmpdir)
        hookmod = types.ModuleType('antenv.axon_hooks')
        hook = _tb._ntff_profile_via_ctypes('/opt/axon/libaxon_pjrt.so')
        hookmod.get_axon_ntff_profile_hook = lambda: hook
        import antenv as _antenv
        sys.modules['antenv.axon_hooks'] = hookmod
        _antenv.axon_hooks = hookmod
        return True
    except Exception:
        return False


def kernel(**inputs) -> np.ndarray:
    import os
    from concourse.bass_utils import run_bass_kernel_spmd
    trace = bool(os.environ.get("KERNEL_TRACE"))
    if trace:
        trace = _enable_trace_shim()

    prep = _host_prep(inputs)
    key = "prog"
    nc = _build_program(prep)

    minvs = np.stack(prep['minv_stacks'], 0)
    in_maps = []
    for c in range(NCORE):
        in_maps.append({
            "AB": prep['AB'], "ATD": prep['ATD'], "AAHD": prep['AAHD'],
            "MINVS": minvs, "WTS": prep['WTS'], "CF": prep['CF'],
            "YBM": np.ascontiguousarray(prep['ybm_cores'][c][:, :512]),
            "YL": prep['ylhsT_cores'][c],
        })
    res = run_bass_kernel_spmd(nc, in_maps, list(range(NCORE)), trace=trace)
    out = np.zeros((B, 2, Nt), np.float32)
    for c in range(NCORE):
        xc = res.results[c]["XOUT"]
        out[c * BS:(c + 1) * BS, 0] = xc[:16]
        out[c * BS:(c + 1) * BS, 1] = xc[16:]
    kernel._last_results = res
    return out



# revision 9
# speedup vs baseline: 1.8326x; 1.8326x over previous
"""DBPNet Trainium2 kernel: 8-core data-parallel Bass/Tile implementation.

v2 scheme:
  - batch-major state [32, N]: row = chan*16 + s (16 samples/core)
  - all heavy matmuls in bf16 (PSUM accum fp32); state/vector math fp32
  - fused ADMM: host-precomputed Q = AAH - AAH*Minv*AAH, T1 = I - AAH*Minv,
    S = I - Minv*AAH  =>  one complex matvec per ADMM step:
      Ax = T1*arc + c*Q*zmu,  x_final = rc1 + A^H*(c*S*zmu - Minv*arc)
  - projection reduced to u' = (1-f)*diff, zmu' = (2f-1)*diff + y
  - A-stacks (both layouts), Q/S and T1/Minv stacks SBUF-resident (bf16)
  - CNN in (co*4+q, (b', l)) layout with block-diagonal weights, bf16 acts;
    BN sums via copy-accum on DVE, sumsq via scalar Square-accum,
    batch stats exact across cores via AllReduce
"""
import numpy as np

B, Nv, Nt, F = 128, 512, 2048, 32
NCORE, BS = 8, 16
ITERS, ADMM = 5, 3
BN_EPS = 1e-5


# ---------------------------------------------------------------- host prep
def _stack_c(M):
    """Complex [512,512] -> comboN rhs stack [128, 8, 512] (f32)."""
    Mr = np.ascontiguousarray(M.real, dtype=np.float32)
    Mi = np.ascontiguousarray(M.imag, dtype=np.float32)
    S1 = np.concatenate([Mr.T, Mi.T], 0)                    # [1024, 512]
    return S1.reshape(8, 128, 512).transpose(1, 0, 2).copy()


def _host_prep(inputs):
    import ml_dtypes
    bf = ml_dtypes.bfloat16

    A = np.ascontiguousarray(np.asarray(inputs['A'], np.float32))
    Ar, Ai = A[0], A[1]
    Ac = Ar.astype(np.float64) + 1j * Ai.astype(np.float64)
    AAH = Ac @ Ac.conj().T
    I = np.eye(Nv)

    rhos = np.exp(np.asarray(inputs['log_rho'], np.float32)).astype(np.float32)
    epss = np.exp(np.asarray(inputs['log_eps'], np.float32)).astype(np.float32)

    qs_stacks, tm_stacks, rho_to_idx, iter_idx = [], [], {}, []
    for r in rhos:
        key = float(r)
        if key not in rho_to_idx:
            Minv = np.linalg.inv(AAH + key * I)
            MA = Minv @ AAH
            Q = AAH - AAH @ MA
            T1 = I - AAH @ Minv
            S = I - MA
            qs_stacks.append(np.concatenate([_stack_c(Q), _stack_c(S)], 1)
                             .astype(bf))                   # [128, 16, 512]
            tm_stacks.append(np.concatenate([_stack_c(T1), _stack_c(Minv)], 1)
                             .astype(bf))
            rho_to_idx[key] = len(qs_stacks) - 1
        iter_idx.append(rho_to_idx[float(r)])

    # A^H-type stack (rows = 2Nv), used with comboH lhsT
    A1 = np.concatenate([Ar, Ai], 0)                        # [1024, 2048]
    AB = A1.reshape(8, 128, 2048).transpose(1, 0, 2).astype(bf).copy()
    # A-type stack (rows = 2Nt), used with comboN(nchunk=16) lhsT
    AT1 = np.concatenate([Ar.T, Ai.T], 0)                   # [4096, 512]
    ATD = AT1.reshape(32, 128, 512).transpose(1, 0, 2).astype(bf).copy()

    w1 = np.asarray(inputs['conv1_w'], np.float32)
    w2 = np.asarray(inputs['conv2_w'], np.float32)
    wf = np.asarray(inputs['convf_w'], np.float32)
    W1 = np.zeros((128, 128), np.float32)
    for dl in range(3):
        for ci in range(2):
            for q in range(4):
                W1[dl * 8 + ci * 4 + q, np.arange(F) * 4 + q] = w1[:, ci, dl]
    W2 = np.zeros((3, 128, 128), np.float32)
    WFm = np.zeros((3, 128, 8), np.float32)
    for dl in range(3):
        for ci in range(F):
            for q in range(4):
                W2[dl, ci * 4 + q, np.arange(F) * 4 + q] = w2[:, ci, dl]
                WFm[dl, ci * 4 + q, np.arange(2) * 4 + q] = wf[:, ci, dl]
    # bf16 weight pack [128, 128 + 384 + 24]
    WTSB = np.concatenate([W1] + [W2[d] for d in range(3)]
                          + [WFm[d] for d in range(3)], axis=1).astype(bf)

    # f32 helper pack: OSEL [32], SELB [128], ID32 [32], ONE1 [1]
    onesel = np.zeros((128, 32), np.float32)
    selback = np.zeros((128, 128), np.float32)
    for co in range(32):
        for q in range(4):
            onesel[co * 4 + q, co] = 1.0
            selback[co, co * 4 + q] = 1.0
    ident32 = np.zeros((128, 32), np.float32)
    ident32[:32, :32] = np.eye(32, dtype=np.float32)
    ones11 = np.zeros((128, 1), np.float32)
    ones11[0, 0] = 1.0
    WTSF = np.concatenate([onesel, selback, ident32, ones11], axis=1)

    g1 = np.asarray(inputs['bn1_g'], np.float32)
    b1 = np.asarray(inputs['bn1_b'], np.float32)
    g2 = np.asarray(inputs['bn2_g'], np.float32)
    b2 = np.asarray(inputs['bn2_b'], np.float32)
    fb = np.asarray(inputs['convf_b'], np.float32)
    # CF cols: 0 g1, 1 b1, 2 g2, 3 b2, 4 zero, 5 eps_bn, 6 tiny, 7.. fbc[it]
    CF = np.zeros((128, 8 + ITERS), np.float32)
    CF[:32, 0] = g1
    CF[:32, 1] = b1
    CF[:32, 2] = g2
    CF[:32, 3] = b2
    CF[:, 5] = BN_EPS
    CF[:, 6] = 1e-30
    for it in range(ITERS):
        c1 = 1.0 / (float(rhos[it]) + 1e-8)
        CF[:16, 7 + it] = fb[0] * c1
        CF[16:32, 7 + it] = fb[1] * c1

    y = np.asarray(inputs['y'], np.float32)
    ybm_cores, ylhsT_cores = [], []
    for c in range(NCORE):
        ys = y[c * BS:(c + 1) * BS]
        ybm = np.concatenate([ys[:, 0], ys[:, 1]], 0)       # [32, Nv]
        ybm_cores.append(np.ascontiguousarray(ybm))
        sT = ybm.T                                          # [Nv, 32]
        bot = np.concatenate([sT[:, 16:], -sT[:, :16]], 1)
        comboH = np.concatenate([sT, bot], 0)               # [2Nv, 32]
        ylhsT_cores.append(
            comboH.reshape(8, 128, 32).transpose(1, 0, 2).astype(bf).copy())

    return dict(AB=AB, ATD=ATD, qs_stacks=qs_stacks, tm_stacks=tm_stacks,
                iter_idx=iter_idx, rhos=rhos, epss=epss,
                WTSB=WTSB, WTSF=WTSF, CF=CF,
                ybm_cores=ybm_cores, ylhsT_cores=ylhsT_cores)


# WTSB column offsets
W1_C = 0
W2_C = 128
WF_C = 128 + 384
WTSB_W = WF_C + 24
# WTSF column offsets
OSEL_C = 0
SELB_C = 32
ID32_C = SELB_C + 128
ONE1_C = ID32_C + 32
WTSF_W = ONE1_C + 1

CF_W = 8 + ITERS


# ---------------------------------------------------------------- program
def _build_program(prep):
    import concourse.bacc as bacc
    import concourse.tile as tile
    import concourse.mybir as mybir

    dt = mybir.dt
    f32, f32r, bf16 = dt.float32, dt.float32r, dt.bfloat16
    AX = mybir.AxisListType
    OP = mybir.AluOpType
    AF = mybir.ActivationFunctionType

    nu = len(prep['qs_stacks'])
    rhos, epss = prep['rhos'], prep['epss']
    cnt = float(B * Nt)

    nc = bacc.Bacc("TRN2", target_bir_lowering=False, debug=False,
                   num_devices=NCORE)

    AB_d = nc.dram_tensor("AB", [128, 8, 2048], bf16, kind="ExternalInput")
    AT_d = nc.dram_tensor("ATD", [128, 32, 512], bf16, kind="ExternalInput")
    QS_d = nc.dram_tensor("QSS", [nu, 128, 16, 512], bf16, kind="ExternalInput")
    TM_d = nc.dram_tensor("TMS", [nu, 128, 16, 512], bf16, kind="ExternalInput")
    WB_d = nc.dram_tensor("WTSB", [128, WTSB_W], bf16, kind="ExternalInput")
    WF_d = nc.dram_tensor("WTSF", [128, WTSF_W], f32r, kind="ExternalInput")
    CF_d = nc.dram_tensor("CF", [128, CF_W], f32, kind="ExternalInput")
    Y_d = nc.dram_tensor("YBM", [32, 512], f32r, kind="ExternalInput")
    YL_d = nc.dram_tensor("YL", [128, 8, 32], bf16, kind="ExternalInput")
    XO_d = nc.dram_tensor("XOUT", [32, 2048], f32r, kind="ExternalOutput")

    with tile.TileContext(nc) as tc:
        with (
            tc.tile_pool(name="cst", bufs=1) as cst,
            tc.tile_pool(name="mats", bufs=1) as matp,
            tc.tile_pool(name="st", bufs=1) as stp,
            tc.tile_pool(name="cmb", bufs=1) as cmb,
            tc.tile_pool(name="act", bufs=5) as actp,
            tc.tile_pool(name="xin", bufs=2) as xinp,
            tc.tile_pool(name="sc", bufs=2) as scp,
            tc.tile_pool(name="jnk", bufs=1) as jnkp,
            tc.tile_pool(name="psB", bufs=3, space="PSUM") as psB,
            tc.tile_pool(name="psA", bufs=3, space="PSUM") as psA,
            tc.tile_pool(name="psT", bufs=1, space="PSUM") as psT,
            tc.tile_pool(name="psC", bufs=1, space="PSUM") as psC,
            tc.tile_pool(name="ddr", bufs=2, space="DRAM") as ddr,
        ):
            # ---- constants into SBUF ----
            ab = cst.tile([128, 8, 2048], bf16, tag="ab")
            at = cst.tile([128, 32, 512], bf16, tag="at")
            wtb = cst.tile([128, WTSB_W], bf16, tag="wtb")
            wtf = cst.tile([128, WTSF_W], f32r, tag="wtf")
            cf = cst.tile([128, CF_W], f32, tag="cf")
            yl = cst.tile([128, 8, 32], bf16, tag="yl")
            nc.sync.dma_start(ab[:], AB_d[:])
            nc.sync.dma_start(at[:], AT_d[:])
            nc.sync.dma_start(wtb[:], WB_d[:])
            nc.sync.dma_start(wtf[:], WF_d[:])
            nc.sync.dma_start(cf[:], CF_d[:])
            nc.sync.dma_start(yl[:], YL_d[:])

            W1 = wtb[0:24, W1_C:W1_C + 128]
            W2 = [wtb[:, W2_C + 128 * d: W2_C + 128 * (d + 1)] for d in range(3)]
            WF = [wtb[:, WF_C + 8 * d: WF_C + 8 * (d + 1)] for d in range(3)]
            OSEL = wtf[:, OSEL_C:OSEL_C + 32]
            SELB = wtf[0:32, SELB_C:SELB_C + 128]
            ID32 = wtf[0:32, ID32_C:ID32_C + 32]
            ONE1 = wtf[0:1, ONE1_C:ONE1_C + 1]
            g32 = [cf[0:32, 0:1], cf[0:32, 2:3]]
            b32 = [cf[0:32, 1:2], cf[0:32, 3:4]]
            zb128 = cf[:, 4:5]
            zb32 = cf[0:32, 4:5]
            zb1 = cf[0:1, 4:5]
            epsb = cf[0:32, 5:6]
            tiny1 = cf[0:1, 6:7]
            fbc = [cf[0:32, 7 + it:8 + it] for it in range(ITERS)]

            # matrix stacks (double-buffered when nu > 1)
            qs_tiles = [matp.tile([128, 16, 512], bf16, tag=f"qs{i}",
                                  name=f"qs{i}")
                        for i in range(min(nu, 2))]
            tm_tiles = [matp.tile([128, 16, 512], bf16, tag=f"tm{i}",
                                  name=f"tm{i}")
                        for i in range(min(nu, 2))]
            slot_of = {}

            def load_slot(uidx, slot):
                nc.sync.dma_start(qs_tiles[slot][:], QS_d[uidx])
                nc.sync.dma_start(tm_tiles[slot][:], TM_d[uidx])
                slot_of[uidx] = slot

            load_slot(prep['iter_idx'][0], 0)
            if nu > 1:
                nxt = next((u for u in prep['iter_idx']
                            if u != prep['iter_idx'][0]), None)
                if nxt is not None:
                    load_slot(nxt, 1)

            # ---- state ----
            x_f = stp.tile([32, 2048], f32r, tag="xf")
            x_b = stp.tile([32, 2048], bf16, tag="xb")
            rc1 = stp.tile([32, 2048], f32r, tag="rc1")
            S8 = stp.tile([32, 8, 512], f32r, tag="S8")
            y_t = S8[:, 0, :]
            u_t = S8[:, 1, :]
            zmu_t = S8[:, 2, :]
            eay0_t = S8[:, 3, :]
            eayu_t = S8[:, 4, :]
            dif_t = S8[:, 5, :]
            marc_t = S8[:, 6, :]
            arc_t = S8[:, 7, :]
            w_t = stp.tile([32, 512], f32r, tag="wt")
            s32f = stp.tile([32, 1], f32, tag="s32f")
            nc.sync.dma_start(y_t[:], Y_d[:])
            nc.vector.memset(u_t[:].bitcast(f32), 0.0)

            zmuT = cmb.tile([128, 8, 32], bf16, tag="zmuT")
            arcT = cmb.tile([128, 8, 32], bf16, tag="arcT")
            wT = cmb.tile([128, 8, 32], bf16, tag="wT")
            rcT = cmb.tile([128, 32, 32], bf16, tag="rcT")
            row1 = cmb.tile([1, 96], f32, tag="row1")
            n2_t = row1[:, 0:16]
            nr_t = row1[:, 16:32]
            f_t = row1[:, 32:48]
            fr1p = row1[:, 48:80]       # (1-f) pair [1,32]
            fr2p_lo = row1[:, 80:96]
            gbt = cmb.tile([32, 2], f32r, tag="gbt")
            gbb = cmb.tile([128, 2], f32, tag="gbb")
            smal = cmb.tile([32, 8], f32r, tag="smal")
            mean_t, var_t = smal[:, 0:1], smal[:, 1:2]
            ssn_t, m2_t = smal[:, 2:3], smal[:, 3:4]
            s1 = cmb.tile([128, 16], f32, tag="s1")
            q1 = cmb.tile([128, 16], f32, tag="q1")
            stat2 = cmb.tile([128, 2], f32, tag="stat2")
            stat2r = cmb.tile([128, 2], f32r, tag="stat2r")
            fr2p = cmb.tile([1, 32], f32, tag="fr2p")
            sc32 = cmb.tile([1, 32], f32, tag="sc32")

            lowp = nc.allow_low_precision

            def comboN4(dst, src):
                """dst [128,8,32] bf16; src [32,512] f32r. slots 0-3 top,
                4-7 = [-si | sr]."""
                pT = psT.tile([128, 4, 32], f32r, tag="tp")
                for c in range(4):
                    nc.tensor.transpose(pT[:, c, :],
                                        src[:, 128 * c:128 * (c + 1)], ID32)
                with lowp(reason="bf16 combo tiles"):
                    nc.vector.tensor_copy(dst[:, 0:4, :], pT[:])
                    nc.vector.tensor_scalar_mul(dst[:, 4:8, 0:16],
                                                pT[:, :, 16:32], -1.0)
                    nc.vector.tensor_copy(dst[:, 4:8, 16:32], pT[:, :, 0:16])

            def comboH4(dst, src):
                """dst [128,8,32] bf16; src [32,512] f32r. slots 0-3 top,
                4-7 = [si | -sr]."""
                pT = psT.tile([128, 4, 32], f32r, tag="tp")
                for c in range(4):
                    nc.tensor.transpose(pT[:, c, :],
                                        src[:, 128 * c:128 * (c + 1)], ID32)
                with lowp(reason="bf16 combo tiles"):
                    nc.vector.tensor_copy(dst[:, 0:4, :], pT[:])
                    nc.vector.tensor_copy(dst[:, 4:8, 0:16], pT[:, :, 16:32])
                    nc.vector.tensor_scalar_mul(dst[:, 4:8, 16:32],
                                                pT[:, :, 0:16], -1.0)

            def comboN16(dst, src):
                """dst [128,32,32] bf16; src [32,2048] f32r."""
                pT = psT.tile([128, 16, 32], f32r, tag="tp")
                for c in range(16):
                    nc.tensor.transpose(pT[:, c, :],
                                        src[:, 128 * c:128 * (c + 1)], ID32)
                with lowp(reason="bf16 combo tiles"):
                    nc.vector.tensor_copy(dst[:, 0:16, :], pT[:])
                    nc.vector.tensor_scalar_mul(dst[:, 16:32, 0:16],
                                                pT[:, :, 16:32], -1.0)
                    nc.vector.tensor_copy(dst[:, 16:32, 16:32], pT[:, :, 0:16])

            # ---- x0 = A^H y ----
            for nt in range(4):
                p = psA.tile([32, 512], f32, tag="mm")
                for k in range(8):
                    nc.tensor.matmul(p[:], yl[:, k, :],
                                     ab[:, k, 512 * nt:512 * (nt + 1)],
                                     start=(k == 0), stop=(k == 7))
                nc.vector.tensor_copy(x_f[:, 512 * nt:512 * (nt + 1)], p[:])
                with lowp(reason="bf16 x copy"):
                    nc.scalar.copy(x_b[:, 512 * nt:512 * (nt + 1)], p[:])

            # ================= iterations =================
            for it in range(ITERS):
                rho = float(rhos[it])
                eps = float(epss[it])
                c1 = 1.0 / (rho + 1e-8)
                cc = rho * c1
                last_it = (it == ITERS - 1)
                uidx = prep['iter_idx'][it]
                if uidx not in slot_of:
                    # evict the slot not needed this iteration
                    other = [s for u, s in slot_of.items()
                             if u != prep['iter_idx'][it - 1]]
                    sl = other[0] if other else 1
                    for u in [u for u, s in slot_of.items() if s == sl]:
                        del slot_of[u]
                    load_slot(uidx, sl)
                qs = qs_tiles[slot_of[uidx]]
                tm = tm_tiles[slot_of[uidx]]

                # ---------- CNN ----------
                def stats_allreduce(layer, acts):
                    with lowp(reason="f32 stat reduce"):
                        nc.vector.tensor_reduce(stat2[:, 0:1], s1[:], AX.X, OP.add)
                        nc.vector.tensor_reduce(stat2[:, 1:2], q1[:], AX.X, OP.add)
                    ci_ = ddr.tile([128, 2], f32, tag="cc")
                    co_ = ddr.tile([128, 2], f32, tag="cc")
                    nc.sync.dma_start(ci_[:], stat2[:])
                    nc.gpsimd.collective_compute(
                        "AllReduce", OP.add,
                        replica_groups=[list(range(NCORE))],
                        ins=[ci_.opt()], outs=[co_.opt()])
                    nc.sync.dma_start(stat2[:], co_[:])
                    with lowp(reason="bn scalar math"):
                        nc.vector.tensor_copy(stat2r[:], stat2[:])
                        p = psC.tile([32, 2], f32, tag="sm")
                        nc.tensor.matmul(p[:], OSEL, stat2r[:],
                                         start=True, stop=True)
                        nc.vector.tensor_scalar_mul(mean_t[:], p[:, 0:1], 1.0 / cnt)
                        nc.vector.tensor_scalar_mul(ssn_t[:], p[:, 1:2], 1.0 / cnt)
                        nc.vector.tensor_mul(m2_t[:], mean_t[:], mean_t[:])
                        nc.vector.tensor_sub(var_t[:], ssn_t[:], m2_t[:])
                        nc.scalar.activation(var_t[:], var_t[:], AF.Sqrt, bias=epsb[:])
                        nc.vector.reciprocal(var_t[:], var_t[:])
                        nc.vector.tensor_mul(gbt[:, 0:1], g32[layer][:], var_t[:])
                        nc.vector.tensor_mul(m2_t[:], mean_t[:], gbt[:, 0:1])
                        nc.vector.tensor_sub(gbt[:, 1:2], b32[layer][:], m2_t[:])
                        p2 = psC.tile([128, 2], f32, tag="sm")
                        nc.tensor.matmul(p2[:], SELB, gbt[:], start=True, stop=True)
                        nc.vector.tensor_copy(gbb[:], p2[:])
                    for a in acts:
                        with lowp(reason="bf16 act"):
                            nc.scalar.activation(a[:, 1:2049], a[:, 1:2049],
                                                 AF.Relu, bias=gbb[:, 1:2],
                                                 scale=gbb[:, 0:1])

                # conv1
                act1 = []
                for bp in range(4):
                    xin = xinp.tile([32, 2048], bf16, tag="xin")
                    nc.vector.memset(xin[0:24, 0:1], 0.0)
                    nc.vector.memset(xin[0:24, 2047:2048], 0.0)
                    for dl in range(3):
                        lo, hi = max(0, 1 - dl), min(2048, 2048 + 1 - dl)
                        for ci in range(2):
                            src = x_b[ci * 16 + bp * 4: ci * 16 + bp * 4 + 4,
                                      lo + dl - 1: hi + dl - 1]
                            nc.sync.dma_start(
                                xin[dl * 8 + ci * 4: dl * 8 + ci * 4 + 4, lo:hi],
                                src)
                    a1 = actp.tile([128, 2050], bf16, tag="act")
                    nc.vector.memset(a1[:, 0:1], 0.0)
                    nc.vector.memset(a1[:, 2049:2050], 0.0)
                    for lt in range(4):
                        p = psB.tile([128, 512], f32, tag="big")
                        nc.tensor.matmul(p[:], W1,
                                         xin[0:24, 512 * lt:512 * (lt + 1)],
                                         start=True, stop=True)
                        idx = bp * 4 + lt
                        with lowp(reason="bf16 act + f32 sums"):
                            nc.vector.tensor_scalar(
                                a1[:, 1 + 512 * lt:1 + 512 * (lt + 1)],
                                p[:], 0.0, 0.0, OP.add, OP.add,
                                accum_out=s1[:, idx:idx + 1])
                        sj = jnkp.tile([128, 512], bf16, tag="sqj")
                        with lowp(reason="sumsq accum"):
                            nc.scalar.activation(
                                sj[:], p[:], AF.Square, bias=zb128[:],
                                accum_out=q1[:, idx:idx + 1])
                    act1.append(a1)
                stats_allreduce(0, act1)

                # conv2
                act2 = []
                for bp in range(4):
                    a2 = actp.tile([128, 2050], bf16, tag="act")
                    nc.vector.memset(a2[:, 0:1], 0.0)
                    nc.vector.memset(a2[:, 2049:2050], 0.0)
                    for lt in range(4):
                        p = psB.tile([128, 512], f32, tag="big")
                        for dl in range(3):
                            nc.tensor.matmul(
                                p[:], W2[dl],
                                act1[bp][:, dl + 512 * lt: dl + 512 * (lt + 1)],
                                start=(dl == 0), stop=(dl == 2))
                        idx = bp * 4 + lt
                        with lowp(reason="bf16 act + f32 sums"):
                            nc.vector.tensor_scalar(
                                a2[:, 1 + 512 * lt:1 + 512 * (lt + 1)],
                                p[:], 0.0, 0.0, OP.add, OP.add,
                                accum_out=s1[:, idx:idx + 1])
                        sj = jnkp.tile([128, 512], bf16, tag="sqj")
                        with lowp(reason="sumsq accum"):
                            nc.scalar.activation(
                                sj[:], p[:], AF.Square, bias=zb128[:],
                                accum_out=q1[:, idx:idx + 1])
                    act2.append(a2)
                stats_allreduce(1, act2)

                # convf -> rc1 (scattered via DMA), then rc1 = c1*(x+conv) + fbc
                for bp in range(4):
                    for lt in range(4):
                        p = psB.tile([8, 512], f32, tag="big")
                        for dl in range(3):
                            nc.tensor.matmul(
                                p[:], WF[dl],
                                act2[bp][:, dl + 512 * lt: dl + 512 * (lt + 1)],
                                start=(dl == 0), stop=(dl == 2))
                        s8 = scp.tile([8, 512], f32r, tag="s8")
                        nc.vector.tensor_copy(s8[:], p[:])
                        for cfi in range(2):
                            nc.sync.dma_start(
                                rc1[cfi * 16 + bp * 4: cfi * 16 + bp * 4 + 4,
                                    512 * lt:512 * (lt + 1)],
                                s8[cfi * 4:cfi * 4 + 4, :])
                nc.vector.tensor_add(rc1[:], rc1[:], x_f[:])
                with lowp(reason="rc1 scale"):
                    nc.scalar.activation(rc1[:], rc1[:], AF.Identity,
                                         bias=fbc[it][:], scale=c1)

                # arc = A * rc1
                comboN16(rcT, rc1)
                parc = psA.tile([32, 512], f32, tag="mm")
                for k in range(32):
                    nc.tensor.matmul(parc[:], rcT[:, k, :], at[:, k, :],
                                     start=(k == 0), stop=(k == 31))
                nc.vector.tensor_copy(arc_t[:], parc[:])

                # e_arc / m_arc = [T1; Minv] * arc
                comboN4(arcT, arc_t)
                pe_ = psA.tile([32, 512], f32, tag="mm")
                for k in range(8):
                    nc.tensor.matmul(pe_[:], arcT[:, k, :], tm[:, k, :],
                                     start=(k == 0), stop=(k == 7))
                pm_ = psA.tile([32, 512], f32, tag="mm")
                for k in range(8):
                    nc.tensor.matmul(pm_[:], arcT[:, k, :], tm[:, 8 + k, :],
                                     start=(k == 0), stop=(k == 7))
                nc.vector.tensor_sub(eay0_t[:], pe_[:], y_t[:])
                nc.vector.tensor_copy(marc_t[:], pm_[:])
                nc.vector.tensor_add(eayu_t[:], eay0_t[:], u_t[:])
                nc.vector.tensor_sub(zmu_t[:], y_t[:], u_t[:])

                # ---------- ADMM ----------
                for s in range(ADMM):
                    final = (s == ADMM - 1)
                    comboN4(zmuT, zmu_t)
                    if not (final and last_it):
                        pq = psA.tile([32, 512], f32, tag="mm")
                        for k in range(8):
                            nc.tensor.matmul(pq[:], zmuT[:, k, :], qs[:, k, :],
                                             start=(k == 0), stop=(k == 7))
                    if final:
                        ps_ = psA.tile([32, 512], f32, tag="mm")
                        for k in range(8):
                            nc.tensor.matmul(ps_[:], zmuT[:, k, :],
                                             qs[:, 8 + k, :],
                                             start=(k == 0), stop=(k == 7))
                    if not (final and last_it):
                        # diff = c*Qzmu + (e_arc - y + u)
                        nc.vector.scalar_tensor_tensor(
                            dif_t[:], pq[:], cc, eayu_t[:], OP.mult, OP.add)
                    if final:
                        # w = c*S*zmu - m_arc ; x = rc1 + A^H w
                        nc.vector.scalar_tensor_tensor(
                            w_t[:], ps_[:], cc, marc_t[:], OP.mult, OP.subtract)
                        comboH4(wT, w_t)
                        for nt in range(4):
                            px = psA.tile([32, 512], f32, tag="mm")
                            for k in range(8):
                                nc.tensor.matmul(
                                    px[:], wT[:, k, :],
                                    ab[:, k, 512 * nt:512 * (nt + 1)],
                                    start=(k == 0), stop=(k == 7))
                            nc.vector.tensor_add(
                                x_f[:, 512 * nt:512 * (nt + 1)], px[:],
                                rc1[:, 512 * nt:512 * (nt + 1)])
                            if not last_it:
                                with lowp(reason="bf16 x copy"):
                                    nc.scalar.copy(
                                        x_b[:, 512 * nt:512 * (nt + 1)],
                                        x_f[:, 512 * nt:512 * (nt + 1)])
                        if last_it:
                            break
                    # ||diff||^2 per row
                    sj2 = jnkp.tile([32, 512], bf16, tag="sqd")
                    with lowp(reason="normsq accum"):
                        nc.scalar.activation(sj2[:], dif_t[:], AF.Square,
                                             bias=zb32[:], accum_out=s32f[:])
                    # factor chain
                    pt = psC.tile([1, 32], f32, tag="sm")
                    nc.tensor.matmul(pt[:], s32f[:], ID32.bitcast(f32),
                                     is_transpose=True)
                    with lowp(reason="factor math"):
                        nc.vector.tensor_copy(sc32[:], pt[:])
                        nc.vector.tensor_add(n2_t[:], sc32[:, 0:16],
                                             sc32[:, 16:32])
                        nc.scalar.activation(nr_t[:], n2_t[:], AF.Sqrt,
                                             bias=tiny1[:],
                                             scale=1.0 / (eps * eps))
                        nc.vector.reciprocal(f_t[:], nr_t[:])
                        nc.vector.tensor_scalar_min(f_t[:], f_t[:], 1.0)
                        # fr1 = 1 - f (both halves), fr2 = 2f - 1
                        nc.vector.tensor_scalar(fr1p[:, 0:16], f_t[:],
                                                -1.0, 1.0, OP.mult, OP.add)
                        nc.vector.tensor_copy(fr1p[:, 16:32], fr1p[:, 0:16])
                        nc.vector.tensor_scalar(fr2p[:], fr1p[:],
                                                -2.0, 1.0, OP.mult, OP.add)
                    pf = psC.tile([32, 2], f32, tag="sm")
                    nc.tensor.matmul(pf[:, 0:1], fr1p, ONE1.bitcast(f32),
                                     is_transpose=True)
                    nc.tensor.matmul(pf[:, 1:2], fr2p[:], ONE1.bitcast(f32),
                                     is_transpose=True)
                    if not final:
                        # zmu' = (2f-1)*diff + y
                        nc.vector.scalar_tensor_tensor(
                            zmu_t[:], dif_t[:], pf[:, 1:2], y_t[:],
                            OP.mult, OP.add)
                    # u' = (1-f)*diff
                    nc.vector.tensor_scalar_mul(u_t[:], dif_t[:], pf[:, 0:1])
                    if not final:
                        nc.vector.tensor_add(eayu_t[:], eay0_t[:], u_t[:])

            nc.sync.dma_start(XO_d[:], x_f[:])

    nc.compile()
    return nc


def _enable_trace_shim():
    import sys, types
    try:
        import trn_agent_boot.trn_boot as _tb
        import concourse.bass_utils as _bu
        _bu.upload_artifacts = lambda tmpdir: "local://" + str(tmpdir)
        hookmod = types.ModuleType('antenv.axon_hooks')
        hook = _tb._ntff_profile_via_ctypes('/opt/axon/libaxon_pjrt.so')
        hookmod.get_axon_ntff_profile_hook = lambda: hook
        import antenv as _antenv
        sys.modules['antenv.axon_hooks'] = hookmod
        _antenv.axon_hooks = hookmod
        return True
    except Exception:
        return False


def kernel(**inputs) -> np.ndarray:
    import os
    from concourse.bass_utils import run_bass_kernel_spmd
    trace = bool(os.environ.get("KERNEL_TRACE"))
    if trace:
        trace = _enable_trace_shim()

    prep = _host_prep(inputs)
    nc = _build_program(prep)

    qss = np.stack(prep['qs_stacks'], 0)
    tms = np.stack(prep['tm_stacks'], 0)
    in_maps = []
    for c in range(NCORE):
        in_maps.append({
            "AB": prep['AB'], "ATD": prep['ATD'],
            "QSS": qss, "TMS": tms,
            "WTSB": prep['WTSB'], "WTSF": prep['WTSF'], "CF": prep['CF'],
            "YBM": np.ascontiguousarray(prep['ybm_cores'][c][:, :512]),
            "YL": prep['ylhsT_cores'][c],
        })
    res = run_bass_kernel_spmd(nc, in_maps, list(range(NCORE)), trace=trace)
    out = np.zeros((B, 2, Nt), np.float32)
    for c in range(NCORE):
        xc = res.results[c]["XOUT"]
        out[c * BS:(c + 1) * BS, 0] = xc[:16]
        out[c * BS:(c + 1) * BS, 1] = xc[16:]
    kernel._last_results = res
    return out
